# revision 1
# baseline (speedup 1.0000x reference)
"""GroupedQueryAttention (B=1, S=4096, D=1024, G=16 heads, DH=64) on 8 TRN2 NeuronCores.

Sharding: tensor-parallel over heads. Core c computes heads {2c, 2c+1}:
  - Q/K/V projections with column-sliced weights (128 out-dims per core),
    producing Q^T/K^T/V^T in [dout, seq] layout (host pre-transposes inputs).
  - Flash-style attention without max-subtraction (scores are tiny:
    |s/8| < ~3), exp on ScalarE with fused 1/8 scale + per-key mask bias.
  - Softmax denominator comes free via an ones-column appended to V in the
    PV matmul (PSUM row 64 accumulates sum_k exp).
  - Output projection with row-sliced Wo produces a partial (4096, 1024)
    output per core; host sums the 8 partials and adds bo.

All matmuls run as float32r (full PE rate at ~fp32 precision).
"""

import os
import sys

for _p in ("/opt/trn_rl_repo", "/root/.axon_site/_ro/trn_rl_repo"):
    if os.path.isdir(_p) and _p not in sys.path:
        sys.path.insert(0, _p)

from contextlib import ExitStack

import numpy as np

import concourse.bass as bass
import concourse.mybir as mybir
import concourse.tile as tile
from concourse import bacc
from concourse.bass_utils import run_bass_kernel_spmd
from concourse.masks import make_identity

S = 4096          # sequence length
D = 1024          # model dim
G = 16            # heads
DH = 64           # head dim
P = 128           # partitions
QT = 512          # q-tile (moving free dim)
KC = 128          # k-chunk
NCORES = 8
HPC = G // NCORES             # heads per core = 2
N_ST = S // QT                # 8 s-tiles of 512
N_KCH = D // P                # 8 contraction chunks for projections
N_KC = S // KC                # 32 k-chunks for attention
DSL = P                       # per-core dout slice (2 heads * 64)

F32 = mybir.dt.float32
F32R = mybir.dt.float32r

_CACHE = {}


def _round_f32r(a):
    """Round fp32 -> fp32r (fp32 with the low 12 mantissa bits dropped)."""
    u = np.ascontiguousarray(a, dtype=np.float32).view(np.uint32)
    u = ((u + np.uint32(0x800)) & np.uint32(0xFFFFF000)).astype(np.uint32)
    return u.view(np.float32)


def _build_nc(dbg=False):
    key = ("nc", dbg)
    if key in _CACHE:
        return _CACHE[key]

    nc = bacc.Bacc(
        "TRN2", target_bir_lowering=False, debug=False, num_devices=NCORES
    )

    xqT = nc.dram_tensor("xqT", [D, S], F32R, kind="ExternalInput").ap()
    xkT = nc.dram_tensor("xkT", [D, S], F32R, kind="ExternalInput").ap()
    xvT = nc.dram_tensor("xvT", [D, S], F32R, kind="ExternalInput").ap()
    wqT = nc.dram_tensor("wqT", [N_KCH, P, DSL], F32R, kind="ExternalInput").ap()
    wkT = nc.dram_tensor("wkT", [N_KCH, P, DSL], F32R, kind="ExternalInput").ap()
    wvT = nc.dram_tensor("wvT", [N_KCH, P, DSL], F32R, kind="ExternalInput").ap()
    woT = nc.dram_tensor("woT", [DSL, D], F32R, kind="ExternalInput").ap()
    bq = nc.dram_tensor("bq", [DSL, 1], F32, kind="ExternalInput").ap()
    bk = nc.dram_tensor("bk", [DSL, 1], F32, kind="ExternalInput").ap()
    bv = nc.dram_tensor("bv", [DSL, 1], F32, kind="ExternalInput").ap()
    mbias = nc.dram_tensor("mbias", [P, N_KC], F32, kind="ExternalInput").ap()
    out_d = nc.dram_tensor("out", [S, D], F32, kind="ExternalOutput").ap()
    if dbg:
        dbg_d = {
            n: nc.dram_tensor(f"dbg_{n}", shp, F32, kind="ExternalOutput").ap()
            for n, shp in (
                ("qts", [P, S]), ("kts", [P, S]), ("vts", [P, S]),
                ("vn0", [P, N_KC * (DH + 4)]), ("vn1", [P, N_KC * (DH + 4)]),
                ("attnT", [P, S]),
            )
        }

    with tile.TileContext(nc) as tc, ExitStack() as ctx:
        consts = ctx.enter_context(tc.tile_pool(name="consts", bufs=1))
        big = ctx.enter_context(tc.tile_pool(name="big", bufs=1))
        xin = ctx.enter_context(tc.tile_pool(name="xin", bufs=6))
        et_pool = ctx.enter_context(tc.tile_pool(name="et", bufs=4))
        small = ctx.enter_context(tc.tile_pool(name="small", bufs=4))
        oevict = ctx.enter_context(tc.tile_pool(name="oevict", bufs=4))
        ps_mm = ctx.enter_context(tc.tile_pool(name="ps_mm", bufs=4, space="PSUM"))
        ps_pv = ctx.enter_context(tc.tile_pool(name="ps_pv", bufs=2, space="PSUM"))
        ps_tr = ctx.enter_context(tc.tile_pool(name="ps_tr", bufs=2, space="PSUM"))

        # ---- constants ----
        ident = consts.tile([P, P], F32)
        make_identity(nc, ident[:])

        w_s = {}
        for name, wd in (("q", wqT), ("k", wkT), ("v", wvT)):
            w = consts.tile([P, N_KCH * DSL], F32R, tag=f"w{name}")
            for kc in range(N_KCH):
                nc.sync.dma_start(w[:, kc * DSL:(kc + 1) * DSL], wd[kc])
            w_s[name] = w
        wo_s = consts.tile([DSL, D], F32R, tag="wo")
        nc.sync.dma_start(wo_s[:], woT)
        b_s = {}
        for name, bd in (("q", bq), ("k", bk), ("v", bv)):
            b = consts.tile([DSL, 1], F32, tag=f"b{name}")
            nc.sync.dma_start(b[:], bd)
            b_s[name] = b
        mb_s = consts.tile([P, N_KC], F32, tag="mb")
        nc.sync.dma_start(mb_s[:], mbias)

        # ---- resident activations ----
        QTs = big.tile([P, S], F32R, tag="QTs")      # Q^T  [dout, s]
        KTs = big.tile([P, S], F32R, tag="KTs")      # K^T  [dout, s]
        VTs = big.tile([P, S], F32, tag="VTs")      # V^T  [dout, s]
        # V natural per head: [k-part, chunk, DH+1]; col DH is the ones
        # column that makes PV also accumulate sum_k exp (softmax denom).
        Vnat = [
            big.tile([P, N_KC, DH + 4], F32R, tag=f"Vn{h}", name=f"Vnat{h}")
            for h in range(HPC)
        ]
        attnT = big.tile([P, S], F32R, tag="attnT")  # normalized attn^T [din, s]

        ones_col = consts.tile([P, 1], F32, tag="ones")
        nc.vector.memset(ones_col[:], 1.0)
        for h in range(HPC):
            nc.vector.tensor_copy(
                Vnat[h][:, :, DH:DH + 1],
                ones_col[:].to_broadcast((P, N_KC, 1)),
            )

        # ---- phase 1: projections (streamed over s-tiles) ----
        for st in range(N_ST):
            sl = slice(st * QT, (st + 1) * QT)
            for name, xd, dst in (("k", xkT, KTs), ("q", xqT, QTs), ("v", xvT, VTs)):
                ps = ps_mm.tile([P, QT], F32, tag="mm")
                for kc in range(N_KCH):
                    xt = xin.tile([P, QT], F32R, tag="xt")
                    nc.sync.dma_start(xt[:], xd[kc * P:(kc + 1) * P, sl])
                    nc.tensor.matmul(
                        ps[:],
                        w_s[name][:, kc * DSL:(kc + 1) * DSL],
                        xt[:],
                        start=(kc == 0),
                        stop=(kc == N_KCH - 1),
                    )
                nc.scalar.activation(
                    dst[:, sl], ps[:],
                    mybir.ActivationFunctionType.Identity,
                    bias=b_s[name][:], scale=1.0,
                )
            # transpose this s-tile of V^T into V natural (4 k-chunks)
            for h in range(HPC):
                hs = slice(h * DH, (h + 1) * DH)
                for j in range(QT // KC):
                    kc = st * (QT // KC) + j
                    pt = ps_tr.tile([P, DH], F32, tag="tr")
                    nc.tensor.transpose(
                        pt[:], VTs[hs, kc * KC:(kc + 1) * KC], ident[hs, hs]
                    )
                    nc.vector.tensor_copy(Vnat[h][:, kc, 0:DH], pt[:])

        # ---- phase 2+3: attention + output projection ----
        for qt in range(N_ST):
            qsl = slice(qt * QT, (qt + 1) * QT)
            for h in range(HPC):
                hs = slice(h * DH, (h + 1) * DH)
                pv = ps_pv.tile([DH + 1, QT], F32, tag="pv")
                for kc in range(N_KC):
                    sc = ps_mm.tile([P, QT], F32, tag="mm")
                    nc.tensor.matmul(
                        sc[:],
                        KTs[hs, kc * KC:(kc + 1) * KC],
                        QTs[hs, qsl],
                        start=True, stop=True,
                    )
                    et = et_pool.tile([P, QT], F32R, tag="et")
                    nc.scalar.activation(
                        et[:], sc[:],
                        mybir.ActivationFunctionType.Exp,
                        bias=mb_s[:, kc:kc + 1], scale=0.125,
                    )
                    nc.tensor.matmul(
                        pv[:],
                        Vnat[h][:, kc, 0:DH + 1],
                        et[:],
                        start=(kc == 0), stop=(kc == N_KC - 1),
                    )
                # normalize: attnT[h, qsl] = pv[0:DH] * (1/pv[DH])
                rec = small.tile([1, QT], F32, tag="rec")
                nc.vector.reciprocal(rec[:], pv[DH:DH + 1, :])
                bc = small.tile([DH, QT], F32, tag="bc")
                nc.gpsimd.partition_broadcast(bc[:], rec[:])
                nc.vector.tensor_mul(attnT[hs, qsl], pv[0:DH, :], bc[:])
            # output projection for the 4 s-tiles of this q-tile
            for j in range(QT // P):
                st = qt * (QT // P) + j
                for nt in range(D // QT):
                    po = ps_mm.tile([P, QT], F32, tag="mm")
                    nc.tensor.matmul(
                        po[:],
                        attnT[:, st * P:(st + 1) * P],
                        wo_s[:, nt * QT:(nt + 1) * QT],
                        start=True, stop=True,
                    )
                    ot = oevict.tile([P, QT], F32, tag="ot")
                    nc.vector.tensor_copy(ot[:], po[:])
                    nc.sync.dma_start(
                        out_d[st * P:(st + 1) * P, nt * QT:(nt + 1) * QT], ot[:]
                    )

        if dbg:
            for name, t in (("qts", QTs), ("kts", KTs), ("vts", VTs),
                            ("attnT", attnT)):
                nc.sync.dma_start(dbg_d[name][:, :], t[:].bitcast(F32))
            nc.sync.dma_start(dbg_d["vn0"][:, :], Vnat[0][:].bitcast(F32))
            nc.sync.dma_start(dbg_d["vn1"][:, :], Vnat[1][:].bitcast(F32))

    nc.compile()
    _CACHE[key] = nc
    return nc


def _prep_in_maps(query, key, value, mask, Wq, bq, Wk, bk, Wv, bv, Wo, bo):
    f = np.float32
    qT = _round_f32r(np.asarray(query)[0].T)
    kT = _round_f32r(np.asarray(key)[0].T)
    vT = _round_f32r(np.asarray(value)[0].T)
    mb = np.where(np.asarray(mask)[0] == 0, f(-1e9), f(0.0)).astype(f)
    mb = np.ascontiguousarray(mb.reshape(N_KC, KC).T)  # [128, 32]
    WqT, WkT, WvT, WoT = (_round_f32r(np.asarray(W).T)
                          for W in (Wq, Wk, Wv, Wo))
    in_maps = []
    for c in range(NCORES):
        cs = slice(c * DSL, (c + 1) * DSL)
        in_maps.append({
            "xqT": qT, "xkT": kT, "xvT": vT,
            "wqT": np.ascontiguousarray(WqT[:, cs]).reshape(N_KCH, P, DSL),
            "wkT": np.ascontiguousarray(WkT[:, cs]).reshape(N_KCH, P, DSL),
            "wvT": np.ascontiguousarray(WvT[:, cs]).reshape(N_KCH, P, DSL),
            "woT": np.ascontiguousarray(WoT[cs, :]),
            "bq": np.ascontiguousarray(bq[cs].astype(f, copy=False)).reshape(DSL, 1),
            "bk": np.ascontiguousarray(bk[cs].astype(f, copy=False)).reshape(DSL, 1),
            "bv": np.ascontiguousarray(bv[cs].astype(f, copy=False)).reshape(DSL, 1),
            "mbias": mb,
        })
    return in_maps


def run(inputs, trace=False, trace_kwargs=None, dbg=False):
    nc = _build_nc(dbg=dbg)
    in_maps = _prep_in_maps(**inputs)
    res = run_bass_kernel_spmd(
        nc, in_maps, core_ids=list(range(NCORES)), trace=trace,
        **(trace_kwargs or {}),
    )
    bo = np.asarray(inputs["bo"], dtype=np.float32)
    acc = np.zeros((S, D), dtype=np.float32)
    for r in res.results:
        acc += r["out"]
    out = (acc + bo[None, :]).astype(np.float32)[None]
    return out, res


def kernel(**inputs):
    out, _ = run(inputs, trace=False)
    return out



# revision 13
# speedup vs baseline: 1.4781x; 1.4781x over previous
"""GroupedQueryAttention (B=1, S=4096, D=1024, G=16 heads, DH=64) on 8 TRN2 NeuronCores.

Sharding: tensor-parallel over heads. Core c computes heads {2c, 2c+1}:
  - Q/K/V projections with column-sliced weights (128 out-dims per core),
    producing Q^T/K^T in [dout, seq] layout (host pre-transposes inputs).
  - V^T is built per head padded to 80 rows with an all-ones row 64; SBUF->
    SBUF xbar DMA transposes turn it into V-natural [kpos, 80] chunks whose
    col 64 is the ones column that makes the PV matmul accumulate the
    softmax denominator for free.
  - Flash-style attention without max-subtraction (scores are tiny:
    |s/8| < ~3), exp on ScalarE with fused 1/8 scale + per-key mask bias.
  - Output projection with row-sliced Wo produces a partial (4096, 1024)
    bf16 output per core; host sums the 8 partials and adds bo.

All matmul operands are bf16 (fp32 PSUM accumulation). K/V inputs stream
through quarter-chunk tiles sized so DMAs stay ahead of the projection
matmuls; Q is loaded and projected per quarter inside the attention phase
so its HBM traffic overlaps compute. Attention runs in (quarter=1024 q,
head) groups whose PV accumulators alternate between two PSUM bank pairs
so the PE stream never gaps at group boundaries (keeps the HAM clock at
K=8/8).
"""

import os
import sys

for _p in ("/opt/trn_rl_repo", "/root/.axon_site/_ro/trn_rl_repo"):
    if os.path.isdir(_p) and _p not in sys.path:
        sys.path.insert(0, _p)

from contextlib import ExitStack

import ml_dtypes
import numpy as np

import concourse.bass as bass
import concourse.mybir as mybir
import concourse.tile as tile
from concourse import bacc
from concourse.bass_utils import run_bass_kernel_spmd

S = 4096          # sequence length
D = 1024          # model dim
G = 16            # heads
DH = 64           # head dim
P = 128           # partitions
QT = 512          # q-tile (moving free dim)
KC = 128          # k-chunk
NCORES = 8
HPC = G // NCORES             # heads per core = 2
N_ST = S // QT                # 8 s-tiles of 512
N_KCH = D // P                # 8 contraction chunks for projections
N_KC = S // KC                # 32 k-chunks for attention
DSL = P                       # per-core dout slice (2 heads * 64)
QPH = 2                       # q-tiles per attention group (quarter)
N_GRP = N_ST // QPH           # 4 quarters
QC = S // N_GRP               # 1024 columns per quarter chunk

F32 = mybir.dt.float32
BF16 = mybir.dt.bfloat16
BF = ml_dtypes.bfloat16

_CACHE = {}


def _build_nc():
    key = "nc"
    if key in _CACHE:
        return _CACHE[key]

    nc = bacc.Bacc(
        "TRN2", target_bir_lowering=False, debug=False, num_devices=NCORES
    )

    xqT = nc.dram_tensor("xqT", [D, S], BF16, kind="ExternalInput").ap()
    xkT = nc.dram_tensor("xkT", [D, S], BF16, kind="ExternalInput").ap()
    xvT = nc.dram_tensor("xvT", [D, S], BF16, kind="ExternalInput").ap()
    wqT = nc.dram_tensor("wqT", [N_KCH, P, DSL], BF16, kind="ExternalInput").ap()
    wkT = nc.dram_tensor("wkT", [N_KCH, P, DSL], BF16, kind="ExternalInput").ap()
    wvT = nc.dram_tensor("wvT", [N_KCH, P, DSL], BF16, kind="ExternalInput").ap()
    woT = nc.dram_tensor("woT", [DSL, D], BF16, kind="ExternalInput").ap()
    bq = nc.dram_tensor("bq", [DSL, 1], F32, kind="ExternalInput").ap()
    bk = nc.dram_tensor("bk", [DSL, 1], F32, kind="ExternalInput").ap()
    bv = nc.dram_tensor("bv", [DSL, 1], F32, kind="ExternalInput").ap()
    mbias = nc.dram_tensor("mbias", [P, N_KC], F32, kind="ExternalInput").ap()
    out_d = nc.dram_tensor("out", [S, D], BF16, kind="ExternalOutput").ap()

    with tile.TileContext(nc) as tc, ExitStack() as ctx:
        consts = ctx.enter_context(tc.tile_pool(name="consts", bufs=1))
        big = ctx.enter_context(tc.tile_pool(name="big", bufs=1))
        et_pool = ctx.enter_context(tc.tile_pool(name="et", bufs=4))
        small = ctx.enter_context(tc.tile_pool(name="small", bufs=2))
        oevict = ctx.enter_context(tc.tile_pool(name="oevict", bufs=4))
        # streamed K/V quarter-chunk tiles: 16 slots per tensor tag = 2
        # quarters of lookahead so chunk DMAs stay ahead of the proj matmuls
        xc = ctx.enter_context(tc.tile_pool(name="xc", bufs=16))
        # streamed Q quarter-chunk tiles, loaded inside the attention phase
        xq = ctx.enter_context(tc.tile_pool(name="xq", bufs=16))
        # PSUM: 'sc' 2 slots x 2 banks + 'pv' 4 slots x 1 bank = 8 banks
        ps_sc = ctx.enter_context(tc.tile_pool(name="ps_sc", bufs=2, space="PSUM"))
        ps_pv = ctx.enter_context(tc.tile_pool(name="ps_pv", bufs=4, space="PSUM"))

        # ---- constants (scalar-queue DMAs, parallel to x loads on sync) ----
        w_s = {}
        for name, wd in (("k", wkT), ("v", wvT), ("q", wqT)):
            w = consts.tile([P, N_KCH * DSL], BF16, tag=f"w{name}")
            for kc in range(N_KCH):
                nc.scalar.dma_start(w[:, kc * DSL:(kc + 1) * DSL], wd[kc])
            w_s[name] = w
        wo_s = consts.tile([DSL, D], BF16, tag="wo")
        nc.scalar.dma_start(wo_s[:], woT)
        b_s = {}
        for name, bd in (("q", bq), ("k", bk), ("v", bv)):
            b = consts.tile([DSL, 1], F32, tag=f"b{name}")
            nc.scalar.dma_start(b[:], bd)
            b_s[name] = b
        mb_s = consts.tile([P, N_KC], F32, tag="mb")
        nc.scalar.dma_start(mb_s[:], mbias)

        def load_quarter(pool, tag, xd, qtr):
            qsl = slice(qtr * QC, (qtr + 1) * QC)
            tiles = []
            for kc in range(N_KCH):
                t = pool.tile([P, QC], BF16, tag=tag, name=f"{tag}{qtr}{kc}")
                nc.sync.dma_start(t[:], xd[kc * P:(kc + 1) * P, qsl])
                tiles.append(t)
            return tiles

        # ---- resident activations ----
        QTs = big.tile([P, S], BF16, tag="QTs")      # Q^T  [dout, s]
        KTs = big.tile([P, S], BF16, tag="KTs")      # K^T  [dout, s]
        # V^T per head, padded to 80 rows: rows 0-63 V data, row 64 ones
        # (becomes the PV ones-column after transpose -> softmax denom),
        # rows 65-79 zero pad (xbar transpose needs partition%16==0).
        VTh = [
            big.tile([DH + 16, S], BF16, tag=f"VTh{h}", name=f"VTh{h}")
            for h in range(HPC)
        ]
        # V natural per head: [k-part, chunk, 80]; col DH is the ones column.
        Vnat = [
            big.tile([P, N_KC, DH + 16], BF16, tag=f"Vn{h}", name=f"Vnat{h}")
            for h in range(HPC)
        ]
        attnT = big.tile([P, S], BF16, tag="attnT")  # normalized attn^T [din, s]

        for h in range(HPC):
            nc.vector.memset(VTh[h][DH:DH + 16, :], 0.0)
            nc.vector.memset(VTh[h][DH:DH + 1, :], 1.0)

        def project(name, dst, st, xtiles, stl):
            """One 512-wide s-tile of the `name` projection into dst (bf16)."""
            sl = slice(st * QT, (st + 1) * QT)
            xsl = slice(stl * QT, (stl + 1) * QT)
            ps = ps_sc.tile([P, QT], F32, tag="sc")
            for kc in range(N_KCH):
                nc.tensor.matmul(
                    ps[:],
                    w_s[name][:, kc * DSL:(kc + 1) * DSL],
                    xtiles[kc][:, xsl],
                    start=(kc == 0),
                    stop=(kc == N_KCH - 1),
                )
            # bias add + bf16 cast on VectorE (keeps ScalarE free for exp)
            nc.vector.tensor_add(
                dst[:, sl], ps[:], b_s[name][:].to_broadcast((P, QT))
            )

        def project_v(st, xtiles, stl):
            """V projection s-tile, split per head into the padded VTh."""
            sl = slice(st * QT, (st + 1) * QT)
            xsl = slice(stl * QT, (stl + 1) * QT)
            ps = ps_sc.tile([P, QT], F32, tag="sc")
            for kc in range(N_KCH):
                nc.tensor.matmul(
                    ps[:],
                    w_s["v"][:, kc * DSL:(kc + 1) * DSL],
                    xtiles[kc][:, xsl],
                    start=(kc == 0),
                    stop=(kc == N_KCH - 1),
                )
            for h in range(HPC):
                hs = slice(h * DH, (h + 1) * DH)
                nc.vector.tensor_add(
                    VTh[h][0:DH, sl], ps[hs, :],
                    b_s["v"][hs, 0:1].to_broadcast((DH, QT)),
                )

        # ---- phase 1: K,V projections; V -> natural via DMA transpose ----
        for qtr in range(N_GRP):
            kt = load_quarter(xc, "xk", xkT, qtr)
            vt = load_quarter(xc, "xv", xvT, qtr)
            for stl in range(QC // QT):
                st = qtr * (QC // QT) + stl
                project("k", KTs, st, kt, stl)
                project_v(st, vt, stl)
                for h in range(HPC):
                    for j in range(QT // KC):
                        kc = st * (QT // KC) + j
                        nc.scalar.dma_start(
                            Vnat[h][:, kc, 0:DH + 16],
                            VTh[h][:, kc * KC:(kc + 1) * KC],
                            transpose=True,
                        )

        # ---- phase 2: Q proj + attention + output projection, per quarter --
        def project_q(qtr, qtiles):
            for stl in range(QC // QT):
                project("q", QTs, qtr * (QC // QT) + stl, qtiles, stl)

        qtiles0 = load_quarter(xq, "xq", xqT, 0)
        project_q(0, qtiles0)
        for grp in range(N_GRP):
            if grp + 1 < N_GRP:
                qtiles_next = load_quarter(xq, "xq", xqT, grp + 1)
            q0 = grp * QPH * QT              # 1024 q-positions per group
            for h in range(HPC):
                hs = slice(h * DH, (h + 1) * DH)
                pvs = [
                    ps_pv.tile([DH + 1, QT], F32, tag="pv", name=f"pv{grp}{h}{j}")
                    for j in range(QPH)
                ]
                for kc in range(N_KC):
                    ks = slice(kc * KC, (kc + 1) * KC)
                    sc = ps_sc.tile([P, QPH * QT], F32, tag="sc")
                    for j in range(QPH):
                        nc.tensor.matmul(
                            sc[:, j * QT:(j + 1) * QT],
                            KTs[hs, ks],
                            QTs[hs, q0 + j * QT:q0 + (j + 1) * QT],
                            start=True, stop=True,
                        )
                    et = et_pool.tile([P, QPH * QT], BF16, tag="et")
                    nc.scalar.activation(
                        et[:], sc[:],
                        mybir.ActivationFunctionType.Exp,
                        bias=mb_s[:, kc:kc + 1], scale=0.125,
                    )
                    for j in range(QPH):
                        nc.tensor.matmul(
                            pvs[j][:],
                            Vnat[h][:, kc, 0:DH + 1],
                            et[:, j * QT:(j + 1) * QT],
                            start=(kc == 0), stop=(kc == N_KC - 1),
                        )
                # normalize: attnT[hs, q] = pv[0:DH] * (1/pv[DH])
                for j in range(QPH):
                    qsl = slice(q0 + j * QT, q0 + (j + 1) * QT)
                    den = small.tile([1, QT], F32, tag="den")
                    nc.vector.tensor_copy(den[:], pvs[j][DH:DH + 1, :])
                    rec = small.tile([1, QT], F32, tag="rec")
                    # approx_fast needs an SBUF source (PSUM input misreads)
                    nc.vector.reciprocal_approx_fast(rec[:], den[:])
                    bc = small.tile([DH, QT], F32, tag="bc")
                    nc.gpsimd.partition_broadcast(bc[:], rec[:])
                    nc.vector.tensor_mul(attnT[hs, qsl], pvs[j][0:DH, :], bc[:])
            # output projection for this quarter's 8 128-wide s-chunks
            for j in range(QPH * QT // P):
                st = grp * (QPH * QT // P) + j
                for nt in range(D // QT):
                    po = ps_pv.tile([P, QT], F32, tag="pv", name=f"po{grp}{j}{nt}")
                    nc.tensor.matmul(
                        po[:],
                        attnT[:, st * P:(st + 1) * P],
                        wo_s[:, nt * QT:(nt + 1) * QT],
                        start=True, stop=True,
                    )
                    ot = oevict.tile([P, QT], BF16, tag="ot")
                    nc.vector.tensor_copy(ot[:], po[:])
                    nc.sync.dma_start(
                        out_d[st * P:(st + 1) * P, nt * QT:(nt + 1) * QT], ot[:]
                    )
            if grp + 1 < N_GRP:
                project_q(grp + 1, qtiles_next)

    nc.compile()
    _CACHE[key] = nc
    return nc


def _prep_in_maps(query, key, value, mask, Wq, bq, Wk, bk, Wv, bv, Wo, bo):
    f = np.float32
    qT = np.ascontiguousarray(np.asarray(query, dtype=f)[0].T).astype(BF)
    kT = np.ascontiguousarray(np.asarray(key, dtype=f)[0].T).astype(BF)
    vT = np.ascontiguousarray(np.asarray(value, dtype=f)[0].T).astype(BF)
    mb = np.where(np.asarray(mask)[0] == 0, f(-1e9), f(0.0)).astype(f)
    mb = np.ascontiguousarray(mb.reshape(N_KC, KC).T)  # [128, 32]
    WqT, WkT, WvT, WoT = (
        np.ascontiguousarray(np.asarray(W, dtype=f).T).astype(BF)
        for W in (Wq, Wk, Wv, Wo)
    )
    in_maps = []
    for c in range(NCORES):
        cs = slice(c * DSL, (c + 1) * DSL)
        in_maps.append({
            "xqT": qT, "xkT": kT, "xvT": vT,
            "wqT": np.ascontiguousarray(WqT[:, cs]).reshape(N_KCH, P, DSL),
            "wkT": np.ascontiguousarray(WkT[:, cs]).reshape(N_KCH, P, DSL),
            "wvT": np.ascontiguousarray(WvT[:, cs]).reshape(N_KCH, P, DSL),
            "woT": np.ascontiguousarray(WoT[cs, :]),
            "bq": np.ascontiguousarray(bq[cs].astype(f, copy=False)).reshape(DSL, 1),
            "bk": np.ascontiguousarray(bk[cs].astype(f, copy=False)).reshape(DSL, 1),
            "bv": np.ascontiguousarray(bv[cs].astype(f, copy=False)).reshape(DSL, 1),
            "mbias": mb,
        })
    return in_maps


def run(inputs, trace=False, trace_kwargs=None):
    nc = _build_nc()
    in_maps = _prep_in_maps(**inputs)
    res = run_bass_kernel_spmd(
        nc, in_maps, core_ids=list(range(NCORES)), trace=trace,
        **(trace_kwargs or {}),
    )
    bo = np.asarray(inputs["bo"], dtype=np.float32)
    acc = np.zeros((S, D), dtype=np.float32)
    for r in res.results:
        acc += np.asarray(r["out"], dtype=np.float32)
    out = (acc + bo[None, :]).astype(np.float32)[None]
    return out, res


def kernel(**inputs):
    out, _ = run(inputs, trace=False)
    return out


# revision 17
# speedup vs baseline: 1.6240x; 1.0987x over previous
"""GroupedQueryAttention (B=1, S=4096, D=1024, G=16 heads, DH=64) on 8 TRN2 NeuronCores.

Sharding: tensor-parallel over heads. Core c computes heads {2c, 2c+1}:
  - Q/K/V projections with column-sliced weights (128 out-dims per core),
    producing Q^T/K^T in [dout, seq] layout (host pre-transposes inputs).
  - V^T is built per head padded to 80 rows with an all-ones row 64; SBUF->
    SBUF xbar DMA transposes turn it into V-natural [kpos, 80] chunks whose
    col 64 is the ones column that makes the PV matmul accumulate the
    softmax denominator for free.
  - Flash-style attention without max-subtraction (scores are tiny:
    |s/8| < ~3), exp on ScalarE with fused 1/8 scale + per-key mask bias.
  - Output projection with row-sliced Wo produces a partial (4096, 1024)
    bf16 output per core; host sums the 8 partials and adds bo.

All matmul operands are bf16 (fp32 PSUM accumulation). K/V inputs stream
through quarter-chunk tiles sized so DMAs stay ahead of the projection
matmuls; Q is loaded and projected per quarter inside the attention phase
so its HBM traffic overlaps compute. Attention runs in (quarter=1024 q,
head) groups whose PV accumulators alternate between two PSUM bank pairs
so the PE stream never gaps at group boundaries (keeps the HAM clock at
K=8/8).
"""

import os
import sys

for _p in ("/opt/trn_rl_repo", "/root/.axon_site/_ro/trn_rl_repo"):
    if os.path.isdir(_p) and _p not in sys.path:
        sys.path.insert(0, _p)

from contextlib import ExitStack

import ml_dtypes
import numpy as np

import concourse.bass as bass
import concourse.mybir as mybir
import concourse.tile as tile
from concourse import bacc
from concourse.bass_utils import run_bass_kernel_spmd

S = 4096          # sequence length
D = 1024          # model dim
G = 16            # heads
DH = 64           # head dim
P = 128           # partitions
QT = 512          # q-tile (moving free dim)
KC = 128          # k-chunk
NCORES = 8
HPC = G // NCORES             # heads per core = 2
N_ST = S // QT                # 8 s-tiles of 512
N_KCH = D // P                # 8 contraction chunks for projections
N_KC = S // KC                # 32 k-chunks for attention
DSL = P                       # per-core dout slice (2 heads * 64)
QPH = 2                       # q-tiles per attention group (quarter)
N_GRP = N_ST // QPH           # 4 quarters
QC = S // N_GRP               # 1024 columns per quarter chunk

F32 = mybir.dt.float32
BF16 = mybir.dt.bfloat16
BF = ml_dtypes.bfloat16

_CACHE = {}


def _build_nc():
    key = "nc"
    if key in _CACHE:
        return _CACHE[key]

    nc = bacc.Bacc(
        "TRN2", target_bir_lowering=False, debug=False, num_devices=NCORES
    )

    xqT = nc.dram_tensor("xqT", [D, S], BF16, kind="ExternalInput").ap()
    xkT = nc.dram_tensor("xkT", [D, S], BF16, kind="ExternalInput").ap()
    xvT = nc.dram_tensor("xvT", [D, S], BF16, kind="ExternalInput").ap()
    wqT = nc.dram_tensor("wqT", [N_KCH, P, DSL], BF16, kind="ExternalInput").ap()
    wkT = nc.dram_tensor("wkT", [N_KCH, P, DSL], BF16, kind="ExternalInput").ap()
    wvT = nc.dram_tensor("wvT", [N_KCH, P, DSL], BF16, kind="ExternalInput").ap()
    woT = nc.dram_tensor("woT", [DSL, D], BF16, kind="ExternalInput").ap()
    bq = nc.dram_tensor("bq", [DSL, 1], F32, kind="ExternalInput").ap()
    bk = nc.dram_tensor("bk", [DSL, 1], F32, kind="ExternalInput").ap()
    bv = nc.dram_tensor("bv", [DSL, 1], F32, kind="ExternalInput").ap()
    mbias = nc.dram_tensor("mbias", [P, N_KC], F32, kind="ExternalInput").ap()
    out_d = nc.dram_tensor("out", [S, D], BF16, kind="ExternalOutput").ap()

    with tile.TileContext(nc) as tc, ExitStack() as ctx:
        consts = ctx.enter_context(tc.tile_pool(name="consts", bufs=1))
        big = ctx.enter_context(tc.tile_pool(name="big", bufs=1))
        et_pool = ctx.enter_context(tc.tile_pool(name="et", bufs=4))
        small = ctx.enter_context(tc.tile_pool(name="small", bufs=2))
        oevict = ctx.enter_context(tc.tile_pool(name="oevict", bufs=4))
        # streamed K/V quarter-chunk tiles: 16 slots per tensor tag = 2
        # quarters of lookahead so chunk DMAs stay ahead of the proj matmuls
        xc = ctx.enter_context(tc.tile_pool(name="xc", bufs=16))
        # streamed Q quarter-chunk tiles, loaded inside the attention phase
        xq = ctx.enter_context(tc.tile_pool(name="xq", bufs=16))
        # PSUM: 'sc' 2 slots x 2 banks + 'pv' 4 slots x 1 bank = 8 banks
        ps_sc = ctx.enter_context(tc.tile_pool(name="ps_sc", bufs=2, space="PSUM"))
        ps_pv = ctx.enter_context(tc.tile_pool(name="ps_pv", bufs=4, space="PSUM"))

        # ---- constants (scalar-queue DMAs, parallel to x loads on sync) ----
        w_s = {}
        for name, wd in (("k", wkT), ("v", wvT), ("q", wqT)):
            w = consts.tile([P, N_KCH * DSL], BF16, tag=f"w{name}")
            for kc in range(N_KCH):
                nc.scalar.dma_start(w[:, kc * DSL:(kc + 1) * DSL], wd[kc])
            w_s[name] = w
        wo_s = consts.tile([DSL, D], BF16, tag="wo")
        nc.scalar.dma_start(wo_s[:], woT)
        b_s = {}
        for name, bd in (("q", bq), ("k", bk), ("v", bv)):
            b = consts.tile([DSL, 1], F32, tag=f"b{name}")
            nc.scalar.dma_start(b[:], bd)
            b_s[name] = b
        mb_s = consts.tile([P, N_KC], F32, tag="mb")
        nc.scalar.dma_start(mb_s[:], mbias)

        def load_quarter(pool, tag, xd, qtr, split=False):
            """Load one quarter's 8 contraction chunks; with split=True the
            chunks alternate between the sync HWDGE and gpsimd SWDGE queues
            so the two DMA paths stream in parallel."""
            qsl = slice(qtr * QC, (qtr + 1) * QC)
            tiles = []
            for kc in range(N_KCH):
                t = pool.tile([P, QC], BF16, tag=tag, name=f"{tag}{qtr}{kc}")
                eng = nc.gpsimd if (split and kc % 2) else nc.sync
                eng.dma_start(t[:], xd[kc * P:(kc + 1) * P, qsl])
                tiles.append(t)
            return tiles

        # ---- resident activations ----
        QTs = big.tile([P, S], BF16, tag="QTs")      # Q^T  [dout, s]
        KTs = big.tile([P, S], BF16, tag="KTs")      # K^T  [dout, s]
        # V^T per head, padded to 80 rows: rows 0-63 V data, row 64 ones
        # (becomes the PV ones-column after transpose -> softmax denom),
        # rows 65-79 zero pad (xbar transpose needs partition%16==0).
        VTh = [
            big.tile([DH + 16, S], BF16, tag=f"VTh{h}", name=f"VTh{h}")
            for h in range(HPC)
        ]
        # V natural per head: [k-part, chunk, 80]; col DH is the ones column.
        Vnat = [
            big.tile([P, N_KC, DH + 16], BF16, tag=f"Vn{h}", name=f"Vnat{h}")
            for h in range(HPC)
        ]
        attnT = big.tile([P, S], BF16, tag="attnT")  # normalized attn^T [din, s]

        for h in range(HPC):
            nc.vector.memset(VTh[h][DH:DH + 16, :], 0.0)
            nc.vector.memset(VTh[h][DH:DH + 1, :], 1.0)

        def project(name, dst, st, xtiles, stl):
            """One 512-wide s-tile of the `name` projection into dst (bf16)."""
            sl = slice(st * QT, (st + 1) * QT)
            xsl = slice(stl * QT, (stl + 1) * QT)
            ps = ps_sc.tile([P, QT], F32, tag="sc")
            for kc in range(N_KCH):
                nc.tensor.matmul(
                    ps[:],
                    w_s[name][:, kc * DSL:(kc + 1) * DSL],
                    xtiles[kc][:, xsl],
                    start=(kc == 0),
                    stop=(kc == N_KCH - 1),
                )
            # bias add + bf16 cast on VectorE (keeps ScalarE free for exp)
            nc.vector.tensor_add(
                dst[:, sl], ps[:], b_s[name][:].to_broadcast((P, QT))
            )

        def project_v(st, xtiles, stl):
            """V projection s-tile, split per head into the padded VTh."""
            sl = slice(st * QT, (st + 1) * QT)
            xsl = slice(stl * QT, (stl + 1) * QT)
            ps = ps_sc.tile([P, QT], F32, tag="sc")
            for kc in range(N_KCH):
                nc.tensor.matmul(
                    ps[:],
                    w_s["v"][:, kc * DSL:(kc + 1) * DSL],
                    xtiles[kc][:, xsl],
                    start=(kc == 0),
                    stop=(kc == N_KCH - 1),
                )
            for h in range(HPC):
                hs = slice(h * DH, (h + 1) * DH)
                nc.vector.tensor_add(
                    VTh[h][0:DH, sl], ps[hs, :],
                    b_s["v"][hs, 0:1].to_broadcast((DH, QT)),
                )

        # ---- phase 1: K,V projections; V -> natural via DMA transpose ----
        qtiles0 = None
        for qtr in range(N_GRP):
            kt = load_quarter(xc, "xk", xkT, qtr, split=True)
            vt = load_quarter(xc, "xv", xvT, qtr, split=True)
            if qtr == 1:
                # prefetch Q quarter 0 while late K/V quarters stream
                qtiles0 = load_quarter(xq, "xq", xqT, 0)
            for stl in range(QC // QT):
                st = qtr * (QC // QT) + stl
                project("k", KTs, st, kt, stl)
                project_v(st, vt, stl)
                for h in range(HPC):
                    for j in range(QT // KC):
                        kc = st * (QT // KC) + j
                        nc.scalar.dma_start(
                            Vnat[h][:, kc, 0:DH + 16],
                            VTh[h][:, kc * KC:(kc + 1) * KC],
                            transpose=True,
                        )

        # ---- phase 2: Q proj + attention + output projection, per quarter --
        def project_q(qtr, qtiles):
            for stl in range(QC // QT):
                project("q", QTs, qtr * (QC // QT) + stl, qtiles, stl)

        project_q(0, qtiles0)
        for grp in range(N_GRP):
            if grp + 1 < N_GRP:
                qtiles_next = load_quarter(xq, "xq", xqT, grp + 1)
            q0 = grp * QPH * QT              # 1024 q-positions per group
            for h in range(HPC):
                hs = slice(h * DH, (h + 1) * DH)
                pvs = [
                    ps_pv.tile([DH + 1, QT], F32, tag="pv", name=f"pv{grp}{h}{j}")
                    for j in range(QPH)
                ]
                for kc in range(N_KC):
                    ks = slice(kc * KC, (kc + 1) * KC)
                    sc = ps_sc.tile([P, QPH * QT], F32, tag="sc")
                    for j in range(QPH):
                        nc.tensor.matmul(
                            sc[:, j * QT:(j + 1) * QT],
                            KTs[hs, ks],
                            QTs[hs, q0 + j * QT:q0 + (j + 1) * QT],
                            start=True, stop=True,
                        )
                    et = et_pool.tile([P, QPH * QT], BF16, tag="et")
                    nc.scalar.activation(
                        et[:], sc[:],
                        mybir.ActivationFunctionType.Exp,
                        bias=mb_s[:, kc:kc + 1], scale=0.125,
                    )
                    for j in range(QPH):
                        nc.tensor.matmul(
                            pvs[j][:],
                            Vnat[h][:, kc, 0:DH + 1],
                            et[:, j * QT:(j + 1) * QT],
                            start=(kc == 0), stop=(kc == N_KC - 1),
                        )
                # normalize: attnT[hs, q] = pv[0:DH] * (1/pv[DH])
                for j in range(QPH):
                    qsl = slice(q0 + j * QT, q0 + (j + 1) * QT)
                    den = small.tile([1, QT], F32, tag="den")
                    nc.vector.tensor_copy(den[:], pvs[j][DH:DH + 1, :])
                    rec = small.tile([1, QT], F32, tag="rec")
                    # approx_fast needs an SBUF source (PSUM input misreads)
                    nc.vector.reciprocal_approx_fast(rec[:], den[:])
                    bc = small.tile([DH, QT], F32, tag="bc")
                    nc.gpsimd.partition_broadcast(bc[:], rec[:])
                    nc.vector.tensor_mul(attnT[hs, qsl], pvs[j][0:DH, :], bc[:])
            # output projection for this quarter's 8 128-wide s-chunks
            for j in range(QPH * QT // P):
                st = grp * (QPH * QT // P) + j
                for nt in range(D // QT):
                    po = ps_pv.tile([P, QT], F32, tag="pv", name=f"po{grp}{j}{nt}")
                    nc.tensor.matmul(
                        po[:],
                        attnT[:, st * P:(st + 1) * P],
                        wo_s[:, nt * QT:(nt + 1) * QT],
                        start=True, stop=True,
                    )
                    ot = oevict.tile([P, QT], BF16, tag="ot")
                    nc.vector.tensor_copy(ot[:], po[:])
                    nc.sync.dma_start(
                        out_d[st * P:(st + 1) * P, nt * QT:(nt + 1) * QT], ot[:]
                    )
            if grp + 1 < N_GRP:
                project_q(grp + 1, qtiles_next)

    nc.compile()
    _CACHE[key] = nc
    return nc


def _prep_in_maps(query, key, value, mask, Wq, bq, Wk, bk, Wv, bv, Wo, bo):
    f = np.float32
    qT = np.ascontiguousarray(np.asarray(query, dtype=f)[0].T).astype(BF)
    kT = np.ascontiguousarray(np.asarray(key, dtype=f)[0].T).astype(BF)
    vT = np.ascontiguousarray(np.asarray(value, dtype=f)[0].T).astype(BF)
    mb = np.where(np.asarray(mask)[0] == 0, f(-1e9), f(0.0)).astype(f)
    mb = np.ascontiguousarray(mb.reshape(N_KC, KC).T)  # [128, 32]
    WqT, WkT, WvT, WoT = (
        np.ascontiguousarray(np.asarray(W, dtype=f).T).astype(BF)
        for W in (Wq, Wk, Wv, Wo)
    )
    in_maps = []
    for c in range(NCORES):
        cs = slice(c * DSL, (c + 1) * DSL)
        in_maps.append({
            "xqT": qT, "xkT": kT, "xvT": vT,
            "wqT": np.ascontiguousarray(WqT[:, cs]).reshape(N_KCH, P, DSL),
            "wkT": np.ascontiguousarray(WkT[:, cs]).reshape(N_KCH, P, DSL),
            "wvT": np.ascontiguousarray(WvT[:, cs]).reshape(N_KCH, P, DSL),
            "woT": np.ascontiguousarray(WoT[cs, :]),
            "bq": np.ascontiguousarray(bq[cs].astype(f, copy=False)).reshape(DSL, 1),
            "bk": np.ascontiguousarray(bk[cs].astype(f, copy=False)).reshape(DSL, 1),
            "bv": np.ascontiguousarray(bv[cs].astype(f, copy=False)).reshape(DSL, 1),
            "mbias": mb,
        })
    return in_maps


def run(inputs, trace=False, trace_kwargs=None):
    nc = _build_nc()
    in_maps = _prep_in_maps(**inputs)
    res = run_bass_kernel_spmd(
        nc, in_maps, core_ids=list(range(NCORES)), trace=trace,
        **(trace_kwargs or {}),
    )
    bo = np.asarray(inputs["bo"], dtype=np.float32)
    acc = np.zeros((S, D), dtype=np.float32)
    for r in res.results:
        acc += np.asarray(r["out"], dtype=np.float32)
    out = (acc + bo[None, :]).astype(np.float32)[None]
    return out, res


def kernel(**inputs):
    out, _ = run(inputs, trace=False)
    return out


# revision 18
# speedup vs baseline: 1.7952x; 1.1054x over previous
"""GroupedQueryAttention (B=1, S=4096, D=1024, G=16 heads, DH=64) on 8 TRN2 NeuronCores.

Sharding: tensor-parallel over heads. Core c computes heads {2c, 2c+1}:
  - Q/K/V projections with column-sliced weights (128 out-dims per core),
    producing Q^T/K^T in [dout, seq] layout (host pre-transposes inputs).
  - V^T is built per head padded to 80 rows with an all-ones row 64; SBUF->
    SBUF xbar DMA transposes turn it into V-natural [kpos, 80] chunks whose
    col 64 is the ones column that makes the PV matmul accumulate the
    softmax denominator for free.
  - Flash-style attention without max-subtraction (scores are tiny:
    |s/8| < ~3), exp on ScalarE with fused 1/8 scale + per-key mask bias.
  - Output projection with row-sliced Wo produces a partial (4096, 1024)
    bf16 output per core; host sums the 8 partials and adds bo.

All matmul operands are bf16 (fp32 PSUM accumulation). K/V inputs stream
through quarter-chunk tiles sized so DMAs stay ahead of the projection
matmuls; Q is loaded and projected per quarter inside the attention phase
so its HBM traffic overlaps compute. Attention runs in (quarter=1024 q,
head) groups whose PV accumulators alternate between two PSUM bank pairs
so the PE stream never gaps at group boundaries (keeps the HAM clock at
K=8/8).
"""

import os
import sys

for _p in ("/opt/trn_rl_repo", "/root/.axon_site/_ro/trn_rl_repo"):
    if os.path.isdir(_p) and _p not in sys.path:
        sys.path.insert(0, _p)

from contextlib import ExitStack

import ml_dtypes
import numpy as np

import concourse.bass as bass
import concourse.mybir as mybir
import concourse.tile as tile
from concourse import bacc
from concourse.bass_utils import run_bass_kernel_spmd

S = 4096          # sequence length
D = 1024          # model dim
G = 16            # heads
DH = 64           # head dim
P = 128           # partitions
QT = 512          # q-tile (moving free dim)
KC = 128          # k-chunk
NCORES = 8
HPC = G // NCORES             # heads per core = 2
N_ST = S // QT                # 8 s-tiles of 512
N_KCH = D // P                # 8 contraction chunks for projections
N_KC = S // KC                # 32 k-chunks for attention
DSL = P                       # per-core dout slice (2 heads * 64)
QPH = 2                       # q-tiles per attention group (quarter)
N_GRP = N_ST // QPH           # 4 quarters
QC = S // N_GRP               # 1024 columns per quarter chunk

F32 = mybir.dt.float32
BF16 = mybir.dt.bfloat16
BF = ml_dtypes.bfloat16

_CACHE = {}


def _build_nc():
    key = "nc"
    if key in _CACHE:
        return _CACHE[key]

    nc = bacc.Bacc(
        "TRN2", target_bir_lowering=False, debug=False, num_devices=NCORES
    )

    xqT = nc.dram_tensor("xqT", [D, S], BF16, kind="ExternalInput").ap()
    xkT = nc.dram_tensor("xkT", [D, S], BF16, kind="ExternalInput").ap()
    xvT = nc.dram_tensor("xvT", [D, S], BF16, kind="ExternalInput").ap()
    wqT = nc.dram_tensor("wqT", [N_KCH, P, DSL], BF16, kind="ExternalInput").ap()
    wkT = nc.dram_tensor("wkT", [N_KCH, P, DSL], BF16, kind="ExternalInput").ap()
    wvT = nc.dram_tensor("wvT", [N_KCH, P, DSL], BF16, kind="ExternalInput").ap()
    woT = nc.dram_tensor("woT", [DSL, D], BF16, kind="ExternalInput").ap()
    bq = nc.dram_tensor("bq", [DSL, 1], F32, kind="ExternalInput").ap()
    bk = nc.dram_tensor("bk", [DSL, 1], F32, kind="ExternalInput").ap()
    bv = nc.dram_tensor("bv", [DSL, 1], F32, kind="ExternalInput").ap()
    mbias = nc.dram_tensor("mbias", [P, N_KC], F32, kind="ExternalInput").ap()
    out_d = nc.dram_tensor("out", [S, D], BF16, kind="ExternalOutput").ap()

    with tile.TileContext(nc) as tc, ExitStack() as ctx:
        consts = ctx.enter_context(tc.tile_pool(name="consts", bufs=1))
        big = ctx.enter_context(tc.tile_pool(name="big", bufs=1))
        et_pool = ctx.enter_context(tc.tile_pool(name="et", bufs=4))
        small = ctx.enter_context(tc.tile_pool(name="small", bufs=2))
        oevict = ctx.enter_context(tc.tile_pool(name="oevict", bufs=4))
        # streamed K/V quarter-chunk tiles: 16 slots per tensor tag = 2
        # quarters of lookahead so chunk DMAs stay ahead of the proj matmuls
        xc = ctx.enter_context(tc.tile_pool(name="xc", bufs=16))
        # streamed Q quarter-chunk tiles, loaded inside the attention phase
        xq = ctx.enter_context(tc.tile_pool(name="xq", bufs=16))
        # PSUM: 'sc' 2 slots x 2 banks + 'pv' 4 slots x 1 bank = 8 banks
        ps_sc = ctx.enter_context(tc.tile_pool(name="ps_sc", bufs=2, space="PSUM"))
        ps_pv = ctx.enter_context(tc.tile_pool(name="ps_pv", bufs=4, space="PSUM"))

        # ---- constants (scalar-queue DMAs, parallel to x loads on sync) ----
        w_s = {}
        for name, wd in (("k", wkT), ("v", wvT), ("q", wqT)):
            w = consts.tile([P, N_KCH * DSL], BF16, tag=f"w{name}")
            for kc in range(N_KCH):
                nc.scalar.dma_start(w[:, kc * DSL:(kc + 1) * DSL], wd[kc])
            w_s[name] = w
        wo_s = consts.tile([DSL, D], BF16, tag="wo")
        nc.scalar.dma_start(wo_s[:], woT)
        b_s = {}
        for name, bd in (("q", bq), ("k", bk), ("v", bv)):
            b = consts.tile([DSL, 1], F32, tag=f"b{name}")
            nc.scalar.dma_start(b[:], bd)
            b_s[name] = b
        mb_s = consts.tile([P, N_KC], F32, tag="mb")
        nc.scalar.dma_start(mb_s[:], mbias)

        def load_quarter(pool, tag, xd, qtr, split=False):
            """Load one quarter's 8 contraction chunks; with split=True the
            chunks alternate between the sync HWDGE and gpsimd SWDGE queues
            so the two DMA paths stream in parallel."""
            qsl = slice(qtr * QC, (qtr + 1) * QC)
            tiles = []
            for kc in range(N_KCH):
                t = pool.tile([P, QC], BF16, tag=tag, name=f"{tag}{qtr}{kc}")
                eng = nc.gpsimd if (split and kc % 2) else nc.sync
                eng.dma_start(t[:], xd[kc * P:(kc + 1) * P, qsl])
                tiles.append(t)
            return tiles

        # ---- resident activations ----
        QTs = big.tile([P, S], BF16, tag="QTs")      # Q^T  [dout, s]
        KTs = big.tile([P, S], BF16, tag="KTs")      # K^T  [dout, s]
        # V^T per head, padded to 80 rows: rows 0-63 V data, row 64 ones
        # (becomes the PV ones-column after transpose -> softmax denom),
        # rows 65-79 zero pad (xbar transpose needs partition%16==0).
        VTh = [
            big.tile([DH + 16, S], BF16, tag=f"VTh{h}", name=f"VTh{h}")
            for h in range(HPC)
        ]
        # V natural per head: [k-part, chunk, 80]; col DH is the ones column.
        Vnat = [
            big.tile([P, N_KC, DH + 16], BF16, tag=f"Vn{h}", name=f"Vnat{h}")
            for h in range(HPC)
        ]
        attnT = big.tile([P, S], BF16, tag="attnT")  # normalized attn^T [din, s]

        for h in range(HPC):
            nc.vector.memset(VTh[h][DH:DH + 16, :], 0.0)
            nc.vector.memset(VTh[h][DH:DH + 1, :], 1.0)

        def project(name, dst, st, xtiles, stl):
            """One 512-wide s-tile of the `name` projection into dst (bf16)."""
            sl = slice(st * QT, (st + 1) * QT)
            xsl = slice(stl * QT, (stl + 1) * QT)
            ps = ps_sc.tile([P, QT], F32, tag="sc")
            for kc in range(N_KCH):
                nc.tensor.matmul(
                    ps[:],
                    w_s[name][:, kc * DSL:(kc + 1) * DSL],
                    xtiles[kc][:, xsl],
                    start=(kc == 0),
                    stop=(kc == N_KCH - 1),
                )
            # bias add + bf16 cast on VectorE (keeps ScalarE free for exp)
            nc.vector.tensor_add(
                dst[:, sl], ps[:], b_s[name][:].to_broadcast((P, QT))
            )

        def project_v(st, xtiles, stl):
            """V projection s-tile, split per head into the padded VTh."""
            sl = slice(st * QT, (st + 1) * QT)
            xsl = slice(stl * QT, (stl + 1) * QT)
            ps = ps_sc.tile([P, QT], F32, tag="sc")
            for kc in range(N_KCH):
                nc.tensor.matmul(
                    ps[:],
                    w_s["v"][:, kc * DSL:(kc + 1) * DSL],
                    xtiles[kc][:, xsl],
                    start=(kc == 0),
                    stop=(kc == N_KCH - 1),
                )
            for h in range(HPC):
                hs = slice(h * DH, (h + 1) * DH)
                nc.vector.tensor_add(
                    VTh[h][0:DH, sl], ps[hs, :],
                    b_s["v"][hs, 0:1].to_broadcast((DH, QT)),
                )

        # ---- phase 1: K,V projections; V -> natural via DMA transpose ----
        qtiles0 = None
        for qtr in range(N_GRP):
            kt = load_quarter(xc, "xk", xkT, qtr, split=True)
            vt = load_quarter(xc, "xv", xvT, qtr, split=True)
            if qtr == 1:
                # prefetch Q quarter 0 while late K/V quarters stream
                qtiles0 = load_quarter(xq, "xq", xqT, 0)
            for stl in range(QC // QT):
                st = qtr * (QC // QT) + stl
                project("k", KTs, st, kt, stl)
                project_v(st, vt, stl)
                # one xbar transpose per (head, s-tile): out chunk j gets
                # k-positions st*512+j*128..+127 at partition s%128
                for h in range(HPC):
                    nc.scalar.dma_start(
                        Vnat[h][:, st * (QT // KC):(st + 1) * (QT // KC), 0:DH + 16],
                        VTh[h][:, st * QT:(st + 1) * QT],
                        transpose=True,
                    )

        # ---- phase 2: Q proj + attention + output projection, per quarter --
        def project_q(qtr, qtiles):
            for stl in range(QC // QT):
                project("q", QTs, qtr * (QC // QT) + stl, qtiles, stl)

        project_q(0, qtiles0)
        for grp in range(N_GRP):
            if grp + 1 < N_GRP:
                qtiles_next = load_quarter(xq, "xq", xqT, grp + 1)
            q0 = grp * QPH * QT              # 1024 q-positions per group
            for h in range(HPC):
                hs = slice(h * DH, (h + 1) * DH)
                pvs = [
                    ps_pv.tile([DH + 1, QT], F32, tag="pv", name=f"pv{grp}{h}{j}")
                    for j in range(QPH)
                ]
                for kc in range(N_KC):
                    ks = slice(kc * KC, (kc + 1) * KC)
                    sc = ps_sc.tile([P, QPH * QT], F32, tag="sc")
                    for j in range(QPH):
                        nc.tensor.matmul(
                            sc[:, j * QT:(j + 1) * QT],
                            KTs[hs, ks],
                            QTs[hs, q0 + j * QT:q0 + (j + 1) * QT],
                            start=True, stop=True,
                        )
                    et = et_pool.tile([P, QPH * QT], BF16, tag="et")
                    nc.scalar.activation(
                        et[:], sc[:],
                        mybir.ActivationFunctionType.Exp,
                        bias=mb_s[:, kc:kc + 1], scale=0.125,
                    )
                    for j in range(QPH):
                        nc.tensor.matmul(
                            pvs[j][:],
                            Vnat[h][:, kc, 0:DH + 1],
                            et[:, j * QT:(j + 1) * QT],
                            start=(kc == 0), stop=(kc == N_KC - 1),
                        )
                # normalize: attnT[hs, q] = pv[0:DH] * (1/pv[DH])
                for j in range(QPH):
                    qsl = slice(q0 + j * QT, q0 + (j + 1) * QT)
                    den = small.tile([1, QT], F32, tag="den")
                    nc.vector.tensor_copy(den[:], pvs[j][DH:DH + 1, :])
                    rec = small.tile([1, QT], F32, tag="rec")
                    # approx_fast needs an SBUF source (PSUM input misreads)
                    nc.vector.reciprocal_approx_fast(rec[:], den[:])
                    bc = small.tile([DH, QT], F32, tag="bc")
                    nc.gpsimd.partition_broadcast(bc[:], rec[:])
                    nc.vector.tensor_mul(attnT[hs, qsl], pvs[j][0:DH, :], bc[:])
            # output projection for this quarter's 8 128-wide s-chunks
            for j in range(QPH * QT // P):
                st = grp * (QPH * QT // P) + j
                for nt in range(D // QT):
                    po = ps_pv.tile([P, QT], F32, tag="pv", name=f"po{grp}{j}{nt}")
                    nc.tensor.matmul(
                        po[:],
                        attnT[:, st * P:(st + 1) * P],
                        wo_s[:, nt * QT:(nt + 1) * QT],
                        start=True, stop=True,
                    )
                    ot = oevict.tile([P, QT], BF16, tag="ot")
                    nc.vector.tensor_copy(ot[:], po[:])
                    nc.sync.dma_start(
                        out_d[st * P:(st + 1) * P, nt * QT:(nt + 1) * QT], ot[:]
                    )
            if grp + 1 < N_GRP:
                project_q(grp + 1, qtiles_next)

    nc.compile()
    _CACHE[key] = nc
    return nc


def _prep_in_maps(query, key, value, mask, Wq, bq, Wk, bk, Wv, bv, Wo, bo):
    f = np.float32
    qT = np.ascontiguousarray(np.asarray(query, dtype=f)[0].T).astype(BF)
    kT = np.ascontiguousarray(np.asarray(key, dtype=f)[0].T).astype(BF)
    vT = np.ascontiguousarray(np.asarray(value, dtype=f)[0].T).astype(BF)
    mb = np.where(np.asarray(mask)[0] == 0, f(-1e9), f(0.0)).astype(f)
    mb = np.ascontiguousarray(mb.reshape(N_KC, KC).T)  # [128, 32]
    WqT, WkT, WvT, WoT = (
        np.ascontiguousarray(np.asarray(W, dtype=f).T).astype(BF)
        for W in (Wq, Wk, Wv, Wo)
    )
    in_maps = []
    for c in range(NCORES):
        cs = slice(c * DSL, (c + 1) * DSL)
        in_maps.append({
            "xqT": qT, "xkT": kT, "xvT": vT,
            "wqT": np.ascontiguousarray(WqT[:, cs]).reshape(N_KCH, P, DSL),
            "wkT": np.ascontiguousarray(WkT[:, cs]).reshape(N_KCH, P, DSL),
            "wvT": np.ascontiguousarray(WvT[:, cs]).reshape(N_KCH, P, DSL),
            "woT": np.ascontiguousarray(WoT[cs, :]),
            "bq": np.ascontiguousarray(bq[cs].astype(f, copy=False)).reshape(DSL, 1),
            "bk": np.ascontiguousarray(bk[cs].astype(f, copy=False)).reshape(DSL, 1),
            "bv": np.ascontiguousarray(bv[cs].astype(f, copy=False)).reshape(DSL, 1),
            "mbias": mb,
        })
    return in_maps


def run(inputs, trace=False, trace_kwargs=None):
    nc = _build_nc()
    in_maps = _prep_in_maps(**inputs)
    res = run_bass_kernel_spmd(
        nc, in_maps, core_ids=list(range(NCORES)), trace=trace,
        **(trace_kwargs or {}),
    )
    bo = np.asarray(inputs["bo"], dtype=np.float32)
    acc = np.zeros((S, D), dtype=np.float32)
    for r in res.results:
        acc += np.asarray(r["out"], dtype=np.float32)
    out = (acc + bo[None, :]).astype(np.float32)[None]
    return out, res


def kernel(**inputs):
    out, _ = run(inputs, trace=False)
    return out
